# revision 25
# baseline (speedup 1.0000x reference)
"""BallQuery Trainium2 kernel, v3: x-sorted spatial windows + PE fp32r
distance matmul + quad-compressed candidate extraction.

Problem: xyz (8, 8192, 3) f32, new_xyz (8, 2048, 3) f32 -> out (8, 2048, 32)
int32.  Per query row (b, m): first 32 point indices j (ascending) with
|q - p_j|^2 < 0.1^2 under f32 reference rounding, padded with the first
valid index; all-sentinel (8193) when none.

Sharding: data-parallel over batch - core b handles batch b.

Host (per batch): sort points and queries by x.  m-tile t covers sorted
queries [128t, 128t+128); its candidate window is a FIXED slice of sorted
points W_t = [lo_t, lo_t + PCAP) with lo_t on a uniform schedule (same for
every core, so the SPMD program is shared).  Any query whose x +/- 0.1 ball
is not covered by its tile window is recomputed exactly on host (zero rows
for uniform data; correctness never depends on the schedule).

Device (per tile): PE fp32r matmul gives psum = |q-p|^2 - r^2 - EPS over
the window (coords zero-centered on host; EPS = 1.8e-3 superset margin
covers the fp32r deviation, so no reference-valid point is missed).  ACT
Sign (or DVE is_lt on some chunks for load balance) marks in-points; one
DVE uint32-pun not_equal collapses point pairs, a second (on GPSIMD)
collapses pairs to quads; a reversed DVE prefix scan (64-zero pad soaks up
the DVE pipeline-warmup glitch) ranks valid quads clamped at 64; GPSIMD
local_scatter compacts quad ids into 64 slots (descending iteration).

Host decode: each filled slot -> 4 sorted positions -> original indices
via the sort permutation; exact f32 recheck of every candidate; sort by
original index; first 32 + reference padding.  Rows whose quad count hit
the 64 clamp are recomputed exactly (rank is by x-position, not index, so
a clamped row may miss low-index candidates).
"""

import numpy as np

import concourse.bacc as bacc
import concourse.bass as bass
import concourse.mybir as mybir
from concourse import bass_utils
from concourse.tile import TileContext

B, N, M = 8, 8192, 2048
NS = 32
K = 5
NT = M // 128  # 16 m-tiles
PCAP = 2304  # candidate window per tile (sorted points)
CHUNKS = (1024, 1024, 256)  # psum chunks
PAIRS = PCAP // 2  # 1152
QUADS = PCAP // 4  # 576
PAD = 128  # zero pad: 3x margin over the observed ~39-elem warmup glitch
CLAMP = 64
NSLOT = 66
OFF = 32768
SENT = N + 1
RADIUS2 = np.float32(0.1) * np.float32(0.1)
EPS = np.float32(1.8e-3)
PUN_SIGN = float(0xBF80BF80)  # bf16 (-1,-1) pair: both out (sign path)
PUN_MASK = 0.0  # (0,0) pair: both out (is_lt path)
BIG = 1 << 30
# chunks handled by DVE is_lt instead of ACT sign, for engine balance
DVE_CHUNKS = 4

# fixed window schedule: tile t center ~ N*(128t+64)/M
LOS = []
for _t in range(NT):
    _lo = int(round((N * (128 * _t + 64) / M - PCAP / 2) / 4.0)) * 4
    LOS.append(max(0, min(N - PCAP, _lo)))

_PLAN = {}


def _build():
    if "nc" in _PLAN:
        return _PLAN["nc"]
    f32 = mybir.dt.float32
    f32r = mybir.dt.float32r
    bf16 = mybir.dt.bfloat16
    i16 = mybir.dt.int16
    u32 = mybir.dt.uint32
    Alu = mybir.AluOpType
    Act = mybir.ActivationFunctionType

    nc = bacc.Bacc("TRN2", target_bir_lowering=False)
    qm_t = nc.dram_tensor("qmat", [K, M], f32r, kind="ExternalInput")
    pm_t = nc.dram_tensor("pmat", [K, N], f32r, kind="ExternalInput")
    out_t = nc.dram_tensor("slots", [M, NSLOT], i16, kind="ExternalOutput")

    # global descending quad descriptor: slice at offset (N/4 - lo/4) of
    # this array, read at scan-output position p, yields quad id
    # (lo/4 + QUADS-1-p) + 1 - OFF + N/4 ... chosen so all values are
    # negative int16: descG[i] = N/4 + QUADS - OFF - i; then
    # quad = value + OFF - 1 ... host decodes quad = val + (OFF - 1) where
    # val = descG[N/4 - lo/4 + p] = lo/4 + QUADS - OFF - p + ...  (see
    # decode below: quad_global = slotval + OFF - 1 - N/4 ... simplified:
    # we store val = quad+1-OFF-N/4?  Keep it simple: descG[i] =
    # (N/4 + QUADS - i) - OFF, slice offset o = N/4 - lo/4, so at position
    # p: val = (lo/4 + QUADS - p) - OFF = quad_id + 1 - OFF with
    # quad_id = lo/4 + QUADS - 1 - p  (mirrored window quad).
    NQ4 = N // 4
    descG = (NQ4 + QUADS - np.arange(NQ4 + QUADS, dtype=np.int64) - OFF).astype(
        np.int16
    )
    descG_d = nc.inline_tensor(
        np.ascontiguousarray(np.broadcast_to(descG, (128, NQ4 + QUADS))),
        name="descG",
    )

    with TileContext(nc) as tc:
        with (
            tc.tile_pool(name="const", bufs=1) as cpool,
            tc.tile_pool(name="sgn", bufs=2) as spool,
            tc.tile_pool(name="pmx", bufs=2) as xpool,
            tc.tile_pool(name="qmx", bufs=2) as qpool,
            tc.tile_pool(name="scan", bufs=2) as ipool,
            tc.psum_pool(name="psA", bufs=3) as ppA,
            tc.psum_pool(name="psB", bufs=2) as ppB,
        ):
            # pmat/qmat use only K=5 partitions: split across DMA queues so
            # the per-partition byte cost parallelizes; issue the slices the
            # first tiles need before the rest so compute starts early.
            qt = cpool.tile([K, M], f32r)
            pt = cpool.tile([K, N], f32r)
            nc.sync.dma_start(qt[:, 0:256], qm_t[:, 0:256])
            for i in range(5):
                nc.sync.dma_start(
                    pt[:, i * 768 : (i + 1) * 768], pm_t[:, i * 768 : (i + 1) * 768]
                )
            for i in range(1, 8):
                nc.sync.dma_start(
                    qt[:, i * 256 : (i + 1) * 256], qm_t[:, i * 256 : (i + 1) * 256]
                )
            for i in range(5, 11):
                lo_i, hi_i = i * 768, min((i + 1) * 768, N)
                if lo_i < hi_i:
                    nc.sync.dma_start(pt[:, lo_i:hi_i], pm_t[:, lo_i:hi_i])
            descs = cpool.tile([128, NQ4 + QUADS], i16)
            half_d = (NQ4 + QUADS) // 2
            for i in range(2):
                nc.sync.dma_start(
                    descs[:, i * half_d : (i + 1) * half_d],
                    descG_d[:, i * half_d : (i + 1) * half_d],
                )
            cC = cpool.tile([128, PAD + QUADS], bf16)
            nc.vector.memset(cC, float(CLAMP))

            dsts = cpool.tile([128, NT * NSLOT], i16)

            ci = 0  # global [1024]-chunk counter for DVE/ACT assignment
            for t in range(NT):
                lo = LOS[t]
                sg = spool.tile([128, PCAP], bf16, tag="sgn")
                coff = 0
                for c, csz in enumerate(CHUNKS):
                    if csz == 1024:
                        ps = ppA.tile([128, 1024], f32, tag="ps")
                        for s in range(2):
                            off = lo + coff + s * 512
                            nc.tensor.matmul(
                                ps[:, s * 512 : (s + 1) * 512],
                                qt[:, t * 128 : (t + 1) * 128],
                                pt[:, off : off + 512],
                            )
                    else:
                        # PSUM allocates whole banks: pad the tile, use 1 bank
                        psb = ppB.tile([128, 512], f32, tag="ps")
                        nc.tensor.matmul(
                            psb[:, 0:csz],
                            qt[:, t * 128 : (t + 1) * 128],
                            pt[:, lo + coff : lo + coff + csz],
                        )
                        ps = psb[:, 0:csz]
                    seg = sg[:, coff : coff + csz]
                    # interleave a few DVE chunks for ACT/DVE balance
                    if csz == 1024 and ci % 8 == 4 and ci // 8 < DVE_CHUNKS:
                        nc.vector.tensor_scalar(seg, ps, 0.0, None, Alu.is_lt)
                        pun_c = PUN_MASK
                    else:
                        nc.scalar.activation(
                            seg, ps[:, :], Act.Sign, bias=0.0, scale=-1.0
                        )
                        pun_c = PUN_SIGN
                    if csz == 1024:
                        ci += 1
                    # pair pun per chunk (constant differs per path)
                    if c == 0:
                        pmx = xpool.tile([128, PAIRS], bf16, tag="pmx")
                    nc.vector.tensor_scalar(
                        pmx[:, coff // 2 : (coff + csz) // 2],
                        seg.bitcast(u32),
                        pun_c,
                        None,
                        Alu.not_equal,
                    )
                    coff += csz

                # quad pun (DVE; the op is not in GPSIMD's ucode set)
                qmx = qpool.tile([128, PAD + QUADS], bf16, tag="qmx")
                if t < 2:
                    nc.vector.memset(qmx[:, 0:PAD], 0.0)
                nc.vector.tensor_scalar(
                    qmx[:, PAD:], pmx[:, :].bitcast(u32), 0.0, None, Alu.not_equal
                )

                sc = ipool.tile([128, PAD + QUADS], i16, tag="scan")
                nc.vector.tensor_tensor_scan(
                    sc[:, ::-1], qmx[:, :], cC[:, :], -1.0, Alu.add, Alu.min
                )

                nc.gpsimd.local_scatter(
                    dsts[:, t * NSLOT : (t + 1) * NSLOT],
                    descs[:, NQ4 - lo // 4 : NQ4 - lo // 4 + QUADS],
                    sc[:, 0:QUADS],
                    channels=128,
                    num_elems=NSLOT,
                    num_idxs=QUADS,
                )
                # stream the output to shorten the tail (finer at the end)
                if t in (3, 7, 11, 13, 14, 15):
                    g = {3: 0, 7: 4, 11: 8, 13: 12, 14: 14, 15: 15}[t]
                    dv = dsts[:, g * NSLOT : (t + 1) * NSLOT].rearrange(
                        "p (t s) -> p t s", s=NSLOT
                    )
                    nc.sync.dma_start(
                        out_t[:]
                        .rearrange("(t p) s -> p t s", p=128)[:, g : t + 1, :],
                        dv,
                    )

    nc.compile()
    _PLAN["nc"] = nc
    return nc


def _prep(xyz_b, new_b, pperm, qperm):
    half = np.float32(0.5)
    ps = (xyz_b[pperm] - half).astype(np.float32)
    qs = (new_b[qperm] - half).astype(np.float32)
    pmat = np.zeros((K, N), dtype=np.float32)
    pmat[0:3] = ps.T
    pmat[3] = (ps * ps).sum(1, dtype=np.float32)
    pmat[4] = 1.0
    qmat = np.zeros((K, M), dtype=np.float32)
    qmat[0:3] = (np.float32(-2.0) * qs).T
    qmat[3] = 1.0
    qmat[4] = (qs * qs).sum(1, dtype=np.float32) - RADIUS2 - EPS
    return pmat, qmat


def _ref_rows(qrows: np.ndarray, pts: np.ndarray) -> np.ndarray:
    """Exact reference for a set of query rows against all points."""
    d = (qrows[:, None, :] - pts[None, :, :]).astype(np.float32)
    sq = (d * d).astype(np.float32)
    s2 = ((sq[..., 0] + sq[..., 1]) + sq[..., 2]).astype(np.float32)
    nq = qrows.shape[0]
    arange = np.broadcast_to(np.arange(N, dtype=np.int64), (nq, N))
    masked = np.where(s2 < RADIUS2, arange, BIG)
    sv = np.sort(masked, axis=1)[:, :NS]
    vals = np.where(sv >= BIG, SENT, sv)
    first = vals[:, 0:1]
    return np.where(vals == SENT, first, vals)


def kernel(xyz: np.ndarray, new_xyz: np.ndarray) -> np.ndarray:
    xyz = np.ascontiguousarray(np.asarray(xyz, dtype=np.float32))
    new_xyz = np.ascontiguousarray(np.asarray(new_xyz, dtype=np.float32))
    nc = _build()

    pperms = np.empty((B, N), dtype=np.int64)
    qperms = np.empty((B, M), dtype=np.int64)
    in_maps = []
    for b in range(B):
        pperms[b] = np.argsort(xyz[b, :, 0], kind="stable")
        qperms[b] = np.argsort(new_xyz[b, :, 0], kind="stable")
        pmat, qmat = _prep(xyz[b], new_xyz[b], pperms[b], qperms[b])
        in_maps.append({"pmat": pmat, "qmat": qmat})

    res = bass_utils.run_bass_kernel_spmd(nc, in_maps, core_ids=list(range(B)))
    slots = np.stack([res.results[b]["slots"] for b in range(B)], axis=0)

    # decode: slot value -> global quad id (sorted space)
    pool = slots[:, :, :CLAMP].astype(np.int64)
    filled = pool != 0
    quad_raw = np.where(filled, pool + (OFF - 1), 0)  # [B, Msorted, CLAMP]
    quad = np.clip(quad_raw, 0, N // 4 - 1)  # crash-proof: bad rows recomputed
    spos = (quad[..., None] * 4 + np.arange(4)).reshape(B, M, CLAMP * 4)
    # original point index via sort permutation
    cand = np.take_along_axis(
        np.broadcast_to(pperms[:, None, :], (B, M, N)), spos, axis=2
    )
    bidx = np.arange(B)[:, None, None]
    gat = xyz[bidx, cand, :]  # [B, Msorted, 256, 3]
    q_s = np.take_along_axis(
        new_xyz, np.broadcast_to(qperms[:, :, None], (B, M, 3)), axis=1
    )
    d = (q_s[:, :, None, :] - gat).astype(np.float32)
    sq = (d * d).astype(np.float32)
    s2 = ((sq[..., 0] + sq[..., 1]) + sq[..., 2]).astype(np.float32)
    keepf = np.repeat(filled, 4, axis=2) & (s2 < RADIUS2)

    masked = np.where(keepf, cand, BIG)
    sv = np.sort(masked, axis=2)[:, :, :NS]
    vals = np.where(sv >= BIG, SENT, sv)
    first = vals[:, :, 0:1]
    out_s = np.where(vals == SENT, first, vals)  # sorted-query order

    # fallback 1: quad pool overflow (rank is x-order, may miss low indices)
    trash = slots[:, :, CLAMP] != 0
    # defensive slot validation: filled slots must be a prefix, strictly
    # increasing, and within the tile's window quad range; violations are
    # recomputed exactly on host (normally zero rows).
    pool16 = slots[:, :, :CLAMP].astype(np.int64)
    fprefix = np.cumsum(pool16 == 0, axis=2) > 0
    hole = ((pool16 != 0) & fprefix).any(axis=2)
    mono = np.zeros((B, M), dtype=bool)
    both = (pool16[:, :, 1:] != 0) & (pool16[:, :, :-1] != 0)
    mono |= (both & (pool16[:, :, 1:] <= pool16[:, :, :-1])).any(axis=2)
    los_t = np.array(LOS, dtype=np.int64) // 4
    lo_per_row = np.repeat(los_t, 128)[None, :]  # [1, M] sorted order
    qv = quad_raw  # unclipped; already masked to 0 where unfilled
    oor = (filled & ((qv < lo_per_row[..., None])
                     | (qv >= lo_per_row[..., None] + QUADS))).any(axis=2)
    trash = trash | hole | mono | oor
    # fallback 2: window coverage violation
    for b in range(B):
        px = xyz[b, pperms[b], 0].astype(np.float64)
        qx = new_xyz[b, qperms[b], 0].astype(np.float64)
        bad = trash[b].copy()
        for t in range(NT):
            lo = LOS[t]
            qs = qx[t * 128 : (t + 1) * 128]
            # coverage: every point with x in [q-0.1-eps, q+0.1+eps] must
            # lie inside [lo, lo+PCAP): the nearest excluded points must be
            # strictly outside that x range.
            viol = np.zeros(128, dtype=bool)
            if lo > 0:
                viol |= px[lo - 1] >= qs - (0.1 + 1e-5)
            if lo + PCAP < N:
                viol |= px[lo + PCAP] <= qs + (0.1 + 1e-5)
            bad[t * 128 : (t + 1) * 128] |= viol
        if bad.any():
            rows = np.where(bad)[0]
            out_s[b, rows] = _ref_rows(
                new_xyz[b, qperms[b][rows]], xyz[b]
            )

    # unpermute queries
    out = np.empty_like(out_s)
    for b in range(B):
        out[b, qperms[b]] = out_s[b]
    return out.astype(np.int32)


if __name__ == "__main__":
    rng = np.random.default_rng(0)
    x = rng.random((B, N, 3), dtype=np.float32)
    q = rng.random((B, M, 3), dtype=np.float32)
    o = kernel(x, q)
    print(o.shape, o.dtype)


# revision 26
# speedup vs baseline: 1.8352x; 1.8352x over previous
"""BallQuery Trainium2 kernel, v3: x-sorted spatial windows + PE fp32r
distance matmul + quad-compressed candidate extraction.

Problem: xyz (8, 8192, 3) f32, new_xyz (8, 2048, 3) f32 -> out (8, 2048, 32)
int32.  Per query row (b, m): first 32 point indices j (ascending) with
|q - p_j|^2 < 0.1^2 under f32 reference rounding, padded with the first
valid index; all-sentinel (8193) when none.

Sharding: data-parallel over batch - core b handles batch b.

Host (per batch): sort points and queries by x.  m-tile t covers sorted
queries [128t, 128t+128); its candidate window is a FIXED slice of sorted
points W_t = [lo_t, lo_t + PCAP) with lo_t on a uniform schedule (same for
every core, so the SPMD program is shared).  Any query whose x +/- 0.1 ball
is not covered by its tile window is recomputed exactly on host (zero rows
for uniform data; correctness never depends on the schedule).

Device (per tile): PE fp32r matmul gives psum = |q-p|^2 - r^2 - EPS over
the window (coords zero-centered on host; EPS = 1.8e-3 superset margin
covers the fp32r deviation, so no reference-valid point is missed).  ACT
Sign (or DVE is_lt on some chunks for load balance) marks in-points; one
DVE uint32-pun not_equal collapses point pairs, a second (on GPSIMD)
collapses pairs to quads; a reversed DVE prefix scan (64-zero pad soaks up
the DVE pipeline-warmup glitch) ranks valid quads clamped at 64; GPSIMD
local_scatter compacts quad ids into 64 slots (descending iteration).

Host decode: each filled slot -> 4 sorted positions -> original indices
via the sort permutation; exact f32 recheck of every candidate; sort by
original index; first 32 + reference padding.  Rows whose quad count hit
the 64 clamp are recomputed exactly (rank is by x-position, not index, so
a clamped row may miss low-index candidates).
"""

import numpy as np

import concourse.bacc as bacc
import concourse.bass as bass
import concourse.mybir as mybir
from concourse import bass_utils
from concourse.tile import TileContext

B, N, M = 8, 8192, 2048
NS = 32
K = 5
NT = M // 128  # 16 m-tiles
PCAP = 2304  # candidate window per tile (sorted points)
CHUNKS = (1024, 1024, 256)  # psum chunks
PAIRS = PCAP // 2  # 1152
QUADS = PCAP // 4  # 576
PAD = 128  # zero pad: 3x margin over the observed ~39-elem warmup glitch
CLAMP = 64
NSLOT = 66
OFF = 32768
SENT = N + 1
RADIUS2 = np.float32(0.1) * np.float32(0.1)
EPS = np.float32(1.8e-3)
PUN_SIGN = float(0xBF80BF80)  # bf16 (-1,-1) pair: both out (sign path)
PUN_MASK = 0.0  # (0,0) pair: both out (is_lt path)
BIG = 1 << 30
# chunks handled by DVE is_lt instead of ACT sign, for engine balance
DVE_CHUNKS = 2

# fixed window schedule: tile t center ~ N*(128t+64)/M
LOS = []
for _t in range(NT):
    _lo = int(round((N * (128 * _t + 64) / M - PCAP / 2) / 4.0)) * 4
    LOS.append(max(0, min(N - PCAP, _lo)))

_PLAN = {}


def _build():
    if "nc" in _PLAN:
        return _PLAN["nc"]
    f32 = mybir.dt.float32
    f32r = mybir.dt.float32r
    bf16 = mybir.dt.bfloat16
    i16 = mybir.dt.int16
    u32 = mybir.dt.uint32
    Alu = mybir.AluOpType
    Act = mybir.ActivationFunctionType

    nc = bacc.Bacc("TRN2", target_bir_lowering=False)
    qm_t = nc.dram_tensor("qmat", [K, M], f32r, kind="ExternalInput")
    pm_t = nc.dram_tensor("pmat", [K, N], f32r, kind="ExternalInput")
    out_t = nc.dram_tensor("slots", [M, NSLOT], i16, kind="ExternalOutput")

    # global descending quad descriptor: slice at offset (N/4 - lo/4) of
    # this array, read at scan-output position p, yields quad id
    # (lo/4 + QUADS-1-p) + 1 - OFF + N/4 ... chosen so all values are
    # negative int16: descG[i] = N/4 + QUADS - OFF - i; then
    # quad = value + OFF - 1 ... host decodes quad = val + (OFF - 1) where
    # val = descG[N/4 - lo/4 + p] = lo/4 + QUADS - OFF - p + ...  (see
    # decode below: quad_global = slotval + OFF - 1 - N/4 ... simplified:
    # we store val = quad+1-OFF-N/4?  Keep it simple: descG[i] =
    # (N/4 + QUADS - i) - OFF, slice offset o = N/4 - lo/4, so at position
    # p: val = (lo/4 + QUADS - p) - OFF = quad_id + 1 - OFF with
    # quad_id = lo/4 + QUADS - 1 - p  (mirrored window quad).
    NQ4 = N // 4
    descG = (NQ4 + QUADS - np.arange(NQ4 + QUADS, dtype=np.int64) - OFF).astype(
        np.int16
    )
    descG_d = nc.inline_tensor(
        np.ascontiguousarray(np.broadcast_to(descG, (128, NQ4 + QUADS))),
        name="descG",
    )

    with TileContext(nc) as tc:
        with (
            tc.tile_pool(name="const", bufs=1) as cpool,
            tc.tile_pool(name="sgn", bufs=2) as spool,
            tc.tile_pool(name="pmx", bufs=2) as xpool,
            tc.tile_pool(name="qmx", bufs=2) as qpool,
            tc.tile_pool(name="scan", bufs=2) as ipool,
            tc.psum_pool(name="psA", bufs=3) as ppA,
            tc.psum_pool(name="psB", bufs=2) as ppB,
        ):
            # pmat/qmat use only K=5 partitions: split across DMA queues so
            # the per-partition byte cost parallelizes; issue the slices the
            # first tiles need before the rest so compute starts early.
            qt = cpool.tile([K, M], f32r)
            pt = cpool.tile([K, N], f32r)
            nc.sync.dma_start(qt[:, 0:256], qm_t[:, 0:256])
            for i in range(5):
                nc.sync.dma_start(
                    pt[:, i * 768 : (i + 1) * 768], pm_t[:, i * 768 : (i + 1) * 768]
                )
            for i in range(1, 8):
                nc.sync.dma_start(
                    qt[:, i * 256 : (i + 1) * 256], qm_t[:, i * 256 : (i + 1) * 256]
                )
            for i in range(5, 11):
                lo_i, hi_i = i * 768, min((i + 1) * 768, N)
                if lo_i < hi_i:
                    nc.sync.dma_start(pt[:, lo_i:hi_i], pm_t[:, lo_i:hi_i])
            descs = cpool.tile([128, NQ4 + QUADS], i16)
            half_d = (NQ4 + QUADS) // 2
            for i in range(2):
                nc.sync.dma_start(
                    descs[:, i * half_d : (i + 1) * half_d],
                    descG_d[:, i * half_d : (i + 1) * half_d],
                )
            cC = cpool.tile([128, PAD + QUADS], bf16)
            nc.vector.memset(cC, float(CLAMP))

            dsts = cpool.tile([128, NT * NSLOT], i16)

            ci = 0  # global [1024]-chunk counter for DVE/ACT assignment
            for t in range(NT):
                lo = LOS[t]
                sg = spool.tile([128, PCAP], bf16, tag="sgn")
                coff = 0
                for c, csz in enumerate(CHUNKS):
                    if csz == 1024:
                        ps = ppA.tile([128, 1024], f32, tag="ps")
                        for s in range(2):
                            off = lo + coff + s * 512
                            nc.tensor.matmul(
                                ps[:, s * 512 : (s + 1) * 512],
                                qt[:, t * 128 : (t + 1) * 128],
                                pt[:, off : off + 512],
                            )
                    else:
                        # PSUM allocates whole banks: pad the tile, use 1 bank
                        psb = ppB.tile([128, 512], f32, tag="ps")
                        nc.tensor.matmul(
                            psb[:, 0:csz],
                            qt[:, t * 128 : (t + 1) * 128],
                            pt[:, lo + coff : lo + coff + csz],
                        )
                        ps = psb[:, 0:csz]
                    seg = sg[:, coff : coff + csz]
                    # interleave a few DVE chunks for ACT/DVE balance
                    if csz == 1024 and ci % 16 == 8 and ci // 16 < DVE_CHUNKS:
                        nc.vector.tensor_scalar(seg, ps, 0.0, None, Alu.is_lt)
                        pun_c = PUN_MASK
                    else:
                        nc.scalar.activation(
                            seg, ps[:, :], Act.Sign, bias=0.0, scale=-1.0
                        )
                        pun_c = PUN_SIGN
                    if csz == 1024:
                        ci += 1
                    # pair pun per chunk (constant differs per path)
                    if c == 0:
                        pmx = xpool.tile([128, PAIRS], bf16, tag="pmx")
                    nc.vector.tensor_scalar(
                        pmx[:, coff // 2 : (coff + csz) // 2],
                        seg.bitcast(u32),
                        pun_c,
                        None,
                        Alu.not_equal,
                    )
                    coff += csz

                # quad pun (DVE; the op is not in GPSIMD's ucode set)
                qmx = qpool.tile([128, PAD + QUADS], bf16, tag="qmx")
                if t < 2:
                    nc.vector.memset(qmx[:, 0:PAD], 0.0)
                nc.vector.tensor_scalar(
                    qmx[:, PAD:], pmx[:, :].bitcast(u32), 0.0, None, Alu.not_equal
                )

                sc = ipool.tile([128, PAD + QUADS], i16, tag="scan")
                nc.vector.tensor_tensor_scan(
                    sc[:, ::-1], qmx[:, :], cC[:, :], -1.0, Alu.add, Alu.min
                )

                nc.gpsimd.local_scatter(
                    dsts[:, t * NSLOT : (t + 1) * NSLOT],
                    descs[:, NQ4 - lo // 4 : NQ4 - lo // 4 + QUADS],
                    sc[:, 0:QUADS],
                    channels=128,
                    num_elems=NSLOT,
                    num_idxs=QUADS,
                )
                # stream the output to shorten the tail (finer at the end)
                if t in (3, 7, 11, 13, 14, 15):
                    g = {3: 0, 7: 4, 11: 8, 13: 12, 14: 14, 15: 15}[t]
                    dv = dsts[:, g * NSLOT : (t + 1) * NSLOT].rearrange(
                        "p (t s) -> p t s", s=NSLOT
                    )
                    nc.sync.dma_start(
                        out_t[:]
                        .rearrange("(t p) s -> p t s", p=128)[:, g : t + 1, :],
                        dv,
                    )

    nc.compile()
    _PLAN["nc"] = nc
    return nc


def _prep(xyz_b, new_b, pperm, qperm):
    half = np.float32(0.5)
    ps = (xyz_b[pperm] - half).astype(np.float32)
    qs = (new_b[qperm] - half).astype(np.float32)
    pmat = np.zeros((K, N), dtype=np.float32)
    pmat[0:3] = ps.T
    pmat[3] = (ps * ps).sum(1, dtype=np.float32)
    pmat[4] = 1.0
    qmat = np.zeros((K, M), dtype=np.float32)
    qmat[0:3] = (np.float32(-2.0) * qs).T
    qmat[3] = 1.0
    qmat[4] = (qs * qs).sum(1, dtype=np.float32) - RADIUS2 - EPS
    return pmat, qmat


def _ref_rows(qrows: np.ndarray, pts: np.ndarray) -> np.ndarray:
    """Exact reference for a set of query rows against all points."""
    d = (qrows[:, None, :] - pts[None, :, :]).astype(np.float32)
    sq = (d * d).astype(np.float32)
    s2 = ((sq[..., 0] + sq[..., 1]) + sq[..., 2]).astype(np.float32)
    nq = qrows.shape[0]
    arange = np.broadcast_to(np.arange(N, dtype=np.int64), (nq, N))
    masked = np.where(s2 < RADIUS2, arange, BIG)
    sv = np.sort(masked, axis=1)[:, :NS]
    vals = np.where(sv >= BIG, SENT, sv)
    first = vals[:, 0:1]
    return np.where(vals == SENT, first, vals)


def kernel(xyz: np.ndarray, new_xyz: np.ndarray) -> np.ndarray:
    xyz = np.ascontiguousarray(np.asarray(xyz, dtype=np.float32))
    new_xyz = np.ascontiguousarray(np.asarray(new_xyz, dtype=np.float32))
    nc = _build()

    pperms = np.empty((B, N), dtype=np.int64)
    qperms = np.empty((B, M), dtype=np.int64)
    in_maps = []
    for b in range(B):
        pperms[b] = np.argsort(xyz[b, :, 0], kind="stable")
        qperms[b] = np.argsort(new_xyz[b, :, 0], kind="stable")
        pmat, qmat = _prep(xyz[b], new_xyz[b], pperms[b], qperms[b])
        in_maps.append({"pmat": pmat, "qmat": qmat})

    res = bass_utils.run_bass_kernel_spmd(nc, in_maps, core_ids=list(range(B)))
    slots = np.stack([res.results[b]["slots"] for b in range(B)], axis=0)

    # decode: slot value -> global quad id (sorted space)
    pool = slots[:, :, :CLAMP].astype(np.int64)
    filled = pool != 0
    quad_raw = np.where(filled, pool + (OFF - 1), 0)  # [B, Msorted, CLAMP]
    quad = np.clip(quad_raw, 0, N // 4 - 1)  # crash-proof: bad rows recomputed
    spos = (quad[..., None] * 4 + np.arange(4)).reshape(B, M, CLAMP * 4)
    # original point index via sort permutation
    cand = np.take_along_axis(
        np.broadcast_to(pperms[:, None, :], (B, M, N)), spos, axis=2
    )
    bidx = np.arange(B)[:, None, None]
    gat = xyz[bidx, cand, :]  # [B, Msorted, 256, 3]
    q_s = np.take_along_axis(
        new_xyz, np.broadcast_to(qperms[:, :, None], (B, M, 3)), axis=1
    )
    d = (q_s[:, :, None, :] - gat).astype(np.float32)
    sq = (d * d).astype(np.float32)
    s2 = ((sq[..., 0] + sq[..., 1]) + sq[..., 2]).astype(np.float32)
    keepf = np.repeat(filled, 4, axis=2) & (s2 < RADIUS2)

    masked = np.where(keepf, cand, BIG)
    sv = np.sort(masked, axis=2)[:, :, :NS]
    vals = np.where(sv >= BIG, SENT, sv)
    first = vals[:, :, 0:1]
    out_s = np.where(vals == SENT, first, vals)  # sorted-query order

    # fallback 1: quad pool overflow (rank is x-order, may miss low indices)
    trash = slots[:, :, CLAMP] != 0
    # defensive slot validation: filled slots must be a prefix, strictly
    # increasing, and within the tile's window quad range; violations are
    # recomputed exactly on host (normally zero rows).
    pool16 = slots[:, :, :CLAMP].astype(np.int64)
    fprefix = np.cumsum(pool16 == 0, axis=2) > 0
    hole = ((pool16 != 0) & fprefix).any(axis=2)
    mono = np.zeros((B, M), dtype=bool)
    both = (pool16[:, :, 1:] != 0) & (pool16[:, :, :-1] != 0)
    mono |= (both & (pool16[:, :, 1:] <= pool16[:, :, :-1])).any(axis=2)
    los_t = np.array(LOS, dtype=np.int64) // 4
    lo_per_row = np.repeat(los_t, 128)[None, :]  # [1, M] sorted order
    qv = quad_raw  # unclipped; already masked to 0 where unfilled
    oor = (filled & ((qv < lo_per_row[..., None])
                     | (qv >= lo_per_row[..., None] + QUADS))).any(axis=2)
    trash = trash | hole | mono | oor
    # fallback 2: window coverage violation
    for b in range(B):
        px = xyz[b, pperms[b], 0].astype(np.float64)
        qx = new_xyz[b, qperms[b], 0].astype(np.float64)
        bad = trash[b].copy()
        for t in range(NT):
            lo = LOS[t]
            qs = qx[t * 128 : (t + 1) * 128]
            # coverage: every point with x in [q-0.1-eps, q+0.1+eps] must
            # lie inside [lo, lo+PCAP): the nearest excluded points must be
            # strictly outside that x range.
            viol = np.zeros(128, dtype=bool)
            if lo > 0:
                viol |= px[lo - 1] >= qs - (0.1 + 1e-5)
            if lo + PCAP < N:
                viol |= px[lo + PCAP] <= qs + (0.1 + 1e-5)
            bad[t * 128 : (t + 1) * 128] |= viol
        if bad.any():
            rows = np.where(bad)[0]
            out_s[b, rows] = _ref_rows(
                new_xyz[b, qperms[b][rows]], xyz[b]
            )

    # unpermute queries
    out = np.empty_like(out_s)
    for b in range(B):
        out[b, qperms[b]] = out_s[b]
    return out.astype(np.int32)


if __name__ == "__main__":
    rng = np.random.default_rng(0)
    x = rng.random((B, N, 3), dtype=np.float32)
    q = rng.random((B, M, 3), dtype=np.float32)
    o = kernel(x, q)
    print(o.shape, o.dtype)
